# revision 58
# baseline (speedup 1.0000x reference)
"""GCN (2-layer) SpMM kernel for 8 TRN2 NeuronCores via Bass/Tile.

Strategy (1D row partitioning, per sharding hint):
  - Destination rows sharded across 8 cores (12500 rows/core, padded to 12544).
  - support1 = x @ W1 computed fully (all nodes) on every core locally,
    stored as an fp16 row-major table in HBM in "padded row space"
    (node n -> row m(n)).  x is supplied pre-transposed + quad-interleaved so
    phase 1 needs no DMA transpose and stores in 512B-contiguous runs.
  - SpMM per layer: edges of a core (contiguous, adj_row sorted) are grouped
    by (window of 128 dest rows, quad-pair parity, row parity) on the host and
    padded to 128-edge tiles.  Per gather batch (<=8 tiles):
      * dma_gather pulls 256B "node pair" elements (2 adjacent fp16 rows,
        at 512B quad stride) from the table into SBUF (edges on partitions),
      * two DVE tensor_tensor ops build all selection matrices of the batch
        at once: S[p, t, j] = val[p,t] * (iota[j] == rowloc[p,t]),
      * one PE matmul per tile accumulates S^T @ G_half into its window PSUM.
    Gather indices are preloaded into SBUF in bulk (no per-gather DMA);
    trailing padding carries idx=-1 so the Q7 firmware skips those
    descriptors (interior padding gathers row 0 with val=0).  An explicit
    depth-8 gather ring keeps ~8 gathers in flight, matching the 4 SWDGE
    queues' ring capacity; a depth-3 ring costs +1.9ms.
  - Bias is added via a K=1 matmul (ones^T @ b) opening each PSUM window.
  - Layer-1 window close: LeakyReLU (ACT Lrelu), PE transpose, h @ W2 ->
    support2 shard; AllGather (fp16) makes the full support2 table; layer 2
    repeats the SpMM and writes fp32 output rows.

Self-contained: hardcodes all shapes; only needs the staged runtime
(concourse) available on the machine, as provided in this container.
"""

import os
import numpy as np

# ---------------------------------------------------------------- config ---


class Cfg:
    def __init__(self, N, E, D=64, NC=8, W=128, SUPER=2, MG=8, IC=8192, QT=8,
                 NQ=4):
        self.N, self.E, self.D, self.NC, self.W, self.SUPER = N, E, D, NC, W, SUPER
        self.R = N // NC                      # real rows per core
        self.NW = -(-self.R // W)             # windows per core
        self.NW = -(-self.NW // SUPER) * SUPER
        self.NSG = self.NW // SUPER
        self.RP = self.NW * W                 # padded rows per core
        self.NPAD = self.RP * NC              # padded table rows
        assert self.NPAD % 512 == 0
        self.NQUAD = self.NPAD // 4           # gather idx = quad index
        assert self.NQUAD <= 32767            # int16 idx
        self.QP = 2                           # pair-within-quad
        self.PAR = 2                          # row-within-pair
        self.MG = MG                          # max tiles per gather instr
        self.IC = IC                          # idx SBUF buffer columns
        self.QT = QT                          # phase-1 512-row blocks per batch
        self.NQ = NQ                          # SWDGE queues
        # MG>8 needs single_packet=False (64-desc packet cap), which makes
        # every 256B descriptor its own packet: measured +0.55ms gpsimd and
        # +0.4ms SWDGE drain at MG=24/NQ=2.  MG=8 + single_packet is best.
        self.RPB = self.RP // 512             # phase-1 full blocks per shard
        self.RTQ = (self.RP % 512) // 4       # tail quads (one 64-quad block)
        assert self.RP % 512 in (0, 256)

    def m_of_node(self, n):
        """node id -> padded table row"""
        return (n // self.R) * self.RP + (n % self.R)


FULL = Cfg(N=100000, E=3200000)
LAST_EXEC_NS = None


# ------------------------------------------------------------- host prep ---


def schedule(cfg, adj_row, adj_col, adj_val):
    """Group each core's edges by (supergroup, quad-parity, row-parity,
    window) and pad each group to whole 128-edge tiles with a schedule shared
    across cores (max tile count per group)."""
    N, NC, W, SUPER = cfg.N, cfg.NC, cfg.W, cfg.SUPER
    QP, PAR, NSG, R = cfg.QP, cfg.PAR, cfg.NSG, cfg.R

    row = np.asarray(adj_row, dtype=np.int64)
    col = np.asarray(adj_col, dtype=np.int64)
    val = np.asarray(adj_val, dtype=np.float32)

    bounds = np.searchsorted(row, np.arange(NC + 1) * R)
    cores = []
    ngroups = NSG * QP * PAR * SUPER
    counts = np.zeros((NC, ngroups), dtype=np.int64)
    for c in range(NC):
        e0, e1 = bounds[c], bounds[c + 1]
        r = (row[e0:e1] - c * R).astype(np.int64)
        m = cfg.m_of_node(col[e0:e1])
        v = val[e0:e1]
        w = r // W
        rowloc = r % W
        quad = m >> 2
        qp = (m >> 1) & 1
        par = m & 1
        sg = w // SUPER
        w4 = w % SUPER
        # emission order: for sg: for qp: for (par, w4) tiles
        key = ((sg * QP + qp) * PAR + par) * SUPER + w4
        order = np.argsort(key, kind="stable")
        cores.append(
            dict(key=key[order], qidx=quad[order], rowloc=rowloc[order], v=v[order])
        )
        counts[c] = np.bincount(key, minlength=ngroups)

    gtiles = -(-counts.max(axis=0) // 128)  # [ngroups]
    tiles = gtiles.reshape(NSG, QP, PAR, SUPER)
    T = max(int(gtiles.sum()), 1)
    return tiles, counts, cores, T


def plan_batches(cfg, tiles):
    """Gather-batch plan, shared by host prep and device program.

    Returns (batches, runs).  Each batch: dict(sg, qp, t0, pc, tmap) where
    tmap[tl] = (par, w4) for each tile in the batch.  Each run: (rt0, rt1)
    tile range whose idx columns are bulk-loaded into one SBUF buffer.
    """
    NSG, QP, PAR, SUPER, MG = cfg.NSG, cfg.QP, cfg.PAR, cfg.SUPER, cfg.MG
    batches = []
    t0 = 0
    for sg in range(NSG):
        for qp in range(QP):
            tmap = []
            for par in range(PAR):
                for w4 in range(SUPER):
                    tmap += [(par, w4)] * int(tiles[sg, qp, par, w4])
            cnt = len(tmap)
            for p0 in range(0, cnt, MG):
                pc = min(MG, cnt - p0)
                batches.append(
                    dict(sg=sg, qp=qp, t0=t0 + p0, pc=pc, tmap=tmap[p0 : p0 + pc])
                )
            t0 += cnt
    # pack batches into idx-buffer runs of <= IC columns (8 cols per tile)
    cap_tiles = cfg.IC // 8
    runs = []
    rt0 = 0
    cur = 0
    for b in batches:
        if b["t0"] + b["pc"] - rt0 > cap_tiles:
            runs.append((rt0, b["t0"]))
            rt0 = b["t0"]
        b["run"] = len(runs)
        cur = b["t0"] + b["pc"]
    runs.append((rt0, max(cur, rt0 + 1)))
    return batches, runs


def prep(cfg, adj_row, adj_col, adj_val):
    """Build the shared tile schedule + per-core edge streams.

    Returns (tiles, batches, runs, per_core list of dicts with
    idxrep [128, T*8] int16, rowloc [128, T] fp16, val [128, T] fp16, T).
    """
    tiles, counts, cores, T = schedule(cfg, adj_row, adj_col, adj_val)
    batches, runs = plan_batches(cfg, tiles)
    gtiles = tiles.reshape(-1)
    ngroups = gtiles.shape[0]
    tstart = np.concatenate([[0], np.cumsum(gtiles)])

    streams = []
    reals = []
    for c in range(cfg.NC):
        d = cores[c]
        idx_s = np.zeros(T * 128, dtype=np.int16)
        rl_s = np.zeros(T * 128, dtype=np.float16)
        vl_s = np.zeros(T * 128, dtype=np.float16)
        real = np.zeros(T * 128, dtype=bool)
        gstart = np.concatenate([[0], np.cumsum(counts[c])])
        for g in range(ngroups):
            cnt = int(counts[c][g])
            if cnt == 0:
                continue
            s0, t0 = gstart[g], tstart[g] * 128
            idx_s[t0 : t0 + cnt] = d["qidx"][s0 : s0 + cnt]
            rl_s[t0 : t0 + cnt] = d["rowloc"][s0 : s0 + cnt]
            vl_s[t0 : t0 + cnt] = d["v"][s0 : s0 + cnt]
            real[t0 : t0 + cnt] = True
        streams.append((idx_s, rl_s, vl_s))
        reals.append(real)

    # Trailing padding of each gather batch -> idx -1: the Q7 firmware trims
    # trailing negatives, skipping their descriptor generation and DMA.
    # num_idxs_reg must equal the non-negative count and is compile-time
    # shared across cores, so the -1 run starts at the max-over-cores last
    # real slot.  Skipped SBUF slots read stale-but-finite data (the gather
    # ring buffers are memset once at start); S has val=0 there.
    for b in batches:
        lo, hi = b["t0"] * 128, (b["t0"] + b["pc"]) * 128
        valid = 1
        for c in range(cfg.NC):
            nz = np.nonzero(reals[c][lo:hi])[0]
            if nz.size:
                valid = max(valid, int(nz[-1]) + 1)
        b["valid"] = valid
        for c in range(cfg.NC):
            streams[c][0][lo + valid : hi] = -1

    per_core = []
    for c in range(cfg.NC):
        idx_s, rl_s, vl_s = streams[c]
        per_core.append(
            dict(
                idxrep=np.broadcast_to(
                    idx_s.reshape(-1, 16).T, (8, 16, T * 8)
                ).reshape(128, T * 8)
                .copy(),                                   # [128, T*8]
                rowloc=rl_s.reshape(T, 128).T.copy(),      # [128, T] fp16
                val=vl_s.reshape(T, 128).T.copy(),         # [128, T] fp16
            )
        )
    return tiles, batches, runs, per_core, T


# --------------------------------------------------------- device program ---


def build_program(cfg, tiles, batches, runs, T, lrelu_native=True):
    import concourse.bass as bass
    import concourse.bacc as bacc
    from concourse import mybir
    from concourse.tile import TileContext

    f16, f32, i16 = mybir.dt.float16, mybir.dt.float32, mybir.dt.int16
    D, W, SUPER, QP, PAR, NSG = cfg.D, cfg.W, cfg.SUPER, cfg.QP, cfg.PAR, cfg.NSG
    NPAD, RP, NQUAD, QT = cfg.NPAD, cfg.RP, cfg.NQUAD, cfg.QT
    SLOTS = T * 8
    RPB, RTQ = cfg.RPB, cfg.RTQ  # per-shard phase-1 blocks

    nc = bacc.Bacc(num_devices=cfg.NC, num_swdge_queues=cfg.NQ,
                   dynamic_dma_scratch_size=65536)

    xtq = nc.declare_dram_parameter("xtq", [D, RP], f16, isOutput=False)
    w1p = nc.declare_dram_parameter("w1", [D, D], f16, isOutput=False)
    w2p = nc.declare_dram_parameter("w2", [D, D], f16, isOutput=False)
    b1p = nc.declare_dram_parameter("b1", [1, D], f16, isOutput=False)
    b2p = nc.declare_dram_parameter("b2", [1, D], f16, isOutput=False)
    idxp = nc.declare_dram_parameter("idxrep", [128, SLOTS], i16, isOutput=False)
    rlp = nc.declare_dram_parameter("rowloc", [128, T], f16, isOutput=False)
    vlp = nc.declare_dram_parameter("val", [128, T], f16, isOutput=False)
    outp = nc.declare_dram_parameter("out", [RP, D], f32, isOutput=True)

    sup1sh = nc.dram_tensor("sup1sh", [RP, D], f16)
    sup1 = nc.dram_tensor("sup1", [NPAD, D], f16, addr_space="Shared")
    s2sh = nc.dram_tensor("s2sh", [RP, D], f16)
    s2full = nc.dram_tensor("s2full", [NPAD, D], f16, addr_space="Shared")

    eq = mybir.AluOpType.is_equal
    mult = mybir.AluOpType.mult

    with TileContext(nc) as tc:
        with (
            tc.tile_pool(name="const", bufs=1) as cp,
            tc.tile_pool(name="meta", bufs=1) as mp,
        ):
            w1s = cp.tile([D, D], f16, tag="w1")
            nc.sync.dma_start(out=w1s[:], in_=w1p[:])
            w2s = cp.tile([D, D], f16, tag="w2")
            nc.sync.dma_start(out=w2s[:], in_=w2p[:])
            b1s = cp.tile([1, D], f16, tag="b1")
            nc.sync.dma_start(out=b1s[:], in_=b1p[:])
            b2s = cp.tile([1, D], f16, tag="b2")
            nc.sync.dma_start(out=b2s[:], in_=b2p[:])
            ones = cp.tile([1, W], f16, tag="ones")
            nc.vector.memset(ones[:], 1.0)
            iota = cp.tile([128, W], f16, tag="iota")
            nc.gpsimd.iota(
                iota[:], [[1, W]], channel_multiplier=0,
                allow_small_or_imprecise_dtypes=True,
            )
            iotap = cp.tile([128, 1], f32, tag="iotap")
            nc.gpsimd.iota(
                iotap[:], [[1, 1]], channel_multiplier=1,
                allow_small_or_imprecise_dtypes=True,
            )
            ident = cp.tile([128, 128], f16, tag="ident")
            nc.vector.tensor_scalar(
                out=ident[:], in0=iota[:, 0:128], scalar1=iotap[:], scalar2=None,
                op0=eq,
            )
            rls = mp.tile([128, T], f16, tag="rl")
            nc.sync.dma_start(out=rls[:], in_=rlp[:])
            vls = mp.tile([128, T], f16, tag="vl")
            nc.sync.dma_start(out=vls[:], in_=vlp[:])

            # ---------------- phase 1: support1 = x @ W1 (own shard) ------
            # Each core computes only its own RP table rows, then an
            # AllGather assembles the full table.  x arrives transposed +
            # quad-interleaved per shard: column b*512 + r*128 + p holds the
            # x row of shard row (b*128 + p)*4 + r, so each psum evacuates
            # into 512B-contiguous quad rows.
            with (
                tc.tile_pool(name="ph1x", bufs=2) as xp,
                tc.tile_pool(name="ph1s", bufs=2) as stp,
                tc.tile_pool(name="ph1ps", bufs=4, space="PSUM") as pp1,
            ):
                for qb in range(0, RPB, QT):
                    nb = min(QT, RPB - qb)
                    xt = xp.tile([D, 512 * nb], f16, tag="xt")
                    nc.sync.dma_start(
                        out=xt[:], in_=xtq[:, qb * 512 : (qb + nb) * 512]
                    )
                    st = stp.tile([128, nb, 256], f16, tag="st")
                    for jj in range(nb):
                        for r4 in range(4):
                            ps = pp1.tile([128, D], f32, tag="ps")
                            nc.tensor.matmul(
                                ps[:],
                                lhsT=xt[:, jj * 512 + r4 * 128 : jj * 512 + (r4 + 1) * 128],
                                rhs=w1s[:],
                                start=True, stop=True,
                            )
                            nc.scalar.activation(
                                out=st[:, jj, r4 * D : (r4 + 1) * D], in_=ps[:],
                                func=mybir.ActivationFunctionType.Copy,
                            )
                    nc.sync.dma_start(
                        out=sup1sh[qb * 512 : (qb + nb) * 512, :].rearrange(
                            "(b p x) d -> p b (x d)", p=128, x=4
                        ),
                        in_=st[:],
                    )
                if RTQ:
                    # 256-row tail block: 64 quads, col r*64 + p
                    xt = xp.tile([D, 4 * RTQ], f16, tag="xtt")
                    nc.sync.dma_start(out=xt[:], in_=xtq[:, RPB * 512 :])
                    st = stp.tile([RTQ, 4 * D], f16, tag="stt")
                    for r4 in range(4):
                        ps = pp1.tile([RTQ, D], f32, tag="pst")
                        nc.tensor.matmul(
                            ps[:],
                            lhsT=xt[:, r4 * RTQ : (r4 + 1) * RTQ],
                            rhs=w1s[:],
                            start=True, stop=True,
                        )
                        nc.scalar.activation(
                            out=st[:, r4 * D : (r4 + 1) * D], in_=ps[:],
                            func=mybir.ActivationFunctionType.Copy,
                        )
                    nc.sync.dma_start(
                        out=sup1sh[RPB * 512 :, :].rearrange(
                            "(p x) d -> p (x d)", x=4
                        ),
                        in_=st[:],
                    )
            nc.gpsimd.collective_compute(
                "AllGather",
                mybir.AluOpType.bypass,
                replica_groups=[list(range(cfg.NC))],
                ins=[sup1sh[:]],
                outs=[sup1[:]],
            )

            # ---------------- SpMM layers --------------------------------
            with (
                tc.tile_pool(name="ib", bufs=2) as ibp,
                tc.tile_pool(name="gp", bufs=8) as gp,
                tc.tile_pool(name="sp", bufs=3) as sp,
                tc.tile_pool(name="hp", bufs=3) as hp,
                tc.tile_pool(name="op", bufs=3) as op,
                tc.tile_pool(name="accp", bufs=5, space="PSUM") as accp,
                tc.tile_pool(name="ptp", bufs=1, space="PSUM") as ptp,
                tc.tile_pool(name="ps2p", bufs=1, space="PSUM") as ps2p,
            ):
                # explicit gather ring: fixed buffers, zeroed once, so slots
                # skipped by trailing -1 idxs always read finite data
                # explicit depth-8 gather ring (same depth as the best pool
                # config, which is what the earlier depth-3 trimming attempt
                # lacked): fixed buffers, zeroed once, so slots skipped by
                # trailing -1 idxs always read finite data
                NGB = 8
                gits = []
                for i in range(NGB):
                    g = gp.tile(
                        [128, cfg.MG, 2 * D], f16, tag=f"G{i}", bufs=1,
                        name=f"gring{i}",
                    )
                    nc.vector.memset(g[:], 0.0)
                    gits.append(g)

                def spmm_layer(layer, table, bias_s):
                    gq = [0]
                    cur_run = [-1, None]  # run id, tile handle

                    def get_idx(b):
                        rid = b["run"]
                        if cur_run[0] != rid:
                            rt0, rt1 = runs[rid]
                            ncols = (rt1 - rt0) * 8
                            ibt = ibp.tile([128, ncols], i16, tag="ib")
                            nc.sync.dma_start(
                                out=ibt[:], in_=idxp[:, rt0 * 8 : rt1 * 8]
                            )
                            cur_run[0], cur_run[1] = rid, ibt
                        rt0 = runs[rid][0]
                        off = (b["t0"] - rt0) * 8
                        return cur_run[1][:, off : off + b["pc"] * 8]

                    bi = 0
                    for sg in range(NSG):
                        left = [
                            int(tiles[sg, :, :, w4].sum()) for w4 in range(SUPER)
                        ]
                        psums = []
                        for w4 in range(SUPER):
                            ps = accp.tile([W, D], f32, tag="acc")
                            nc.tensor.matmul(
                                ps[:], lhsT=ones[:], rhs=bias_s[:],
                                start=True, stop=(left[w4] == 0),
                            )
                            psums.append(ps)
                        while bi < len(batches) and batches[bi]["sg"] == sg:
                            b = batches[bi]
                            pc, qp, t0 = b["pc"], b["qp"], b["t0"]
                            idx_ap = get_idx(b)
                            git = gits[gq[0] % NGB][:, :pc, :]
                            nc.gpsimd.dma_gather(
                                git,
                                bass.AP(
                                    table, qp * 2 * D,
                                    [[4 * D, NQUAD], [1, 2 * D]],
                                ),
                                idx_ap,
                                num_idxs=pc * 128,
                                num_idxs_reg=b["valid"],
                                elem_size=2 * D,
                                elem_step=4 * D,
                                single_packet=cfg.MG <= 8,
                                queue_num=gq[0] % cfg.NQ,
                            )
                            gq[0] += 1
                            seq = sp.tile([128, pc, W], f16, tag="Seq")
                            nc.vector.tensor_tensor(
                                out=seq[:],
                                in0=iota[:].unsqueeze(1).broadcast_to([128, pc, W]),
                                in1=rls[:, t0 : t0 + pc]
                                .unsqueeze(2)
                                .broadcast_to([128, pc, W]),
                                op=eq,
                            )
                            Ss = sp.tile([128, pc, W], f16, tag="S")
                            nc.vector.tensor_tensor(
                                out=Ss[:],
                                in0=seq[:],
                                in1=vls[:, t0 : t0 + pc]
                                .unsqueeze(2)
                                .broadcast_to([128, pc, W]),
                                op=mult,
                            )
                            for tl in range(pc):
                                par, w4 = b["tmap"][tl]
                                left[w4] -= 1
                                nc.tensor.matmul(
                                    psums[w4][:],
                                    lhsT=Ss[:, tl, :],
                                    rhs=git[:, tl, par * D : par * D + D],
                                    start=False,
                                    stop=(left[w4] == 0),
                                )
                            bi += 1
                        # window close
                        if layer == 0:
                            s2t = hp.tile([W, SUPER, D], f16, tag="s2t")
                        else:
                            s2t = op.tile([W, SUPER, D], f32, tag="outt")
                        for w4 in range(SUPER):
                            if layer == 0:
                                hh = hp.tile([W, D], f16, tag="hh")
                                if lrelu_native:
                                    nc.scalar.activation(
                                        out=hh[:], in_=psums[w4][:],
                                        func=mybir.ActivationFunctionType.Lrelu,
                                        alpha=0.2,
                                    )
                                else:
                                    hpos = hp.tile([W, D], f16, tag="hpos")
                                    nc.scalar.activation(
                                        out=hpos[:], in_=psums[w4][:],
                                        func=mybir.ActivationFunctionType.Relu,
                                    )
                                    hneg = hp.tile([W, D], f16, tag="hneg")
                                    nc.vector.tensor_scalar(
                                        out=hneg[:], in0=psums[w4][:],
                                        scalar1=0.0, scalar2=0.2,
                                        op0=mybir.AluOpType.min, op1=mult,
                                    )
                                    nc.vector.tensor_tensor(
                                        out=hh[:], in0=hpos[:], in1=hneg[:],
                                        op=mybir.AluOpType.add,
                                    )
                                pt = ptp.tile([D, W], f16, tag="pt")
                                nc.tensor.transpose(pt[:], hh[:], ident[:])
                                hT = hp.tile([D, W], f16, tag="hT")
                                nc.scalar.activation(
                                    out=hT[:], in_=pt[:],
                                    func=mybir.ActivationFunctionType.Copy,
                                )
                                ps2 = ps2p.tile([W, D], f32, tag="ps2")
                                nc.tensor.matmul(
                                    ps2[:], lhsT=hT[:], rhs=w2s[:],
                                    start=True, stop=True,
                                )
                                nc.scalar.activation(
                                    out=s2t[:, w4, :], in_=ps2[:],
                                    func=mybir.ActivationFunctionType.Copy,
                                )
                            else:
                                nc.scalar.activation(
                                    out=s2t[:, w4, :], in_=psums[w4][:],
                                    func=mybir.ActivationFunctionType.Copy,
                                )
                        dst = s2sh if layer == 0 else outp
                        nc.sync.dma_start(
                            out=dst[
                                sg * SUPER * W : (sg + 1) * SUPER * W, :
                            ].rearrange("(t p) d -> p t d", p=W),
                            in_=s2t[:],
                        )

                spmm_layer(0, sup1, b1s)
                nc.gpsimd.collective_compute(
                    "AllGather",
                    mybir.AluOpType.bypass,
                    replica_groups=[list(range(cfg.NC))],
                    ins=[s2sh[:]],
                    outs=[s2full[:]],
                )
                spmm_layer(1, s2full, b2s)

    nc.compile()
    return nc


# ----------------------------------------------------------------- kernel ---


def make_inputs(cfg, x, adj_row, adj_col, adj_val, W1, b1, W2, b2):
    tiles, batches, runs, per_core, T = prep(cfg, adj_row, adj_col, adj_val)
    x = np.asarray(x, dtype=np.float32)
    xpad = np.zeros((cfg.NPAD, cfg.D), dtype=np.float16)
    xpad[cfg.m_of_node(np.arange(cfg.N))] = x.astype(np.float16)
    common = dict(
        w1=np.asarray(W1, np.float16),
        w2=np.asarray(W2, np.float16),
        b1=np.asarray(b1, np.float16).reshape(1, cfg.D),
        b2=np.asarray(b2, np.float16).reshape(1, cfg.D),
    )
    in_maps = []
    for c in range(cfg.NC):
        m = dict(common)
        # shard-local transpose + quad interleave: col b*512 + r*128 + p
        # (tail: RPB*512 + r*RTQ + p) <- shard row (quad)*4 + r
        xl = xpad[c * cfg.RP : (c + 1) * cfg.RP]
        xf = (
            xl[: cfg.RPB * 512]
            .reshape(cfg.RPB, 128, 4, cfg.D)
            .transpose(0, 2, 1, 3)
            .reshape(-1, cfg.D)
        )
        if cfg.RTQ:
            xt = (
                xl[cfg.RPB * 512 :]
                .reshape(cfg.RTQ, 4, cfg.D)
                .transpose(1, 0, 2)
                .reshape(-1, cfg.D)
            )
            xf = np.concatenate([xf, xt], axis=0)
        m["xtq"] = np.ascontiguousarray(xf.T)      # [D, RP]
        m["idxrep"] = per_core[c]["idxrep"]
        m["rowloc"] = per_core[c]["rowloc"]
        m["val"] = per_core[c]["val"]
        in_maps.append(m)
    return tiles, batches, runs, in_maps, T


def kernel(x, adj_row, adj_col, adj_val, W1, b1, W2, b2, _cfg=None, _sim=False):
    cfg = _cfg or FULL
    tiles, batches, runs, in_maps, T = make_inputs(
        cfg, x, adj_row, adj_col, adj_val, W1, b1, W2, b2
    )
    # Native ACT Lrelu mis-evaluates on HW (rel err 0.2 observed); keep the
    # Relu + min/mult + add composition on both paths.
    nc = build_program(cfg, tiles, batches, runs, T, lrelu_native=False)
    if _sim:
        from concourse import bass_interp

        sim = bass_interp.MultiCoreSim(nc, cfg.NC)
        for c in range(cfg.NC):
            for k, v in in_maps[c].items():
                sim.cores[c].tensor(k)[:] = v
        sim.simulate()
        results = [{"out": np.array(sim.cores[c].tensor("out"))} for c in range(cfg.NC)]
    else:
        from concourse.bass_utils import run_bass_kernel_spmd

        trace = bool(int(os.environ.get("GCN_TRACE", "0")))
        res = run_bass_kernel_spmd(nc, in_maps, list(range(cfg.NC)), trace=trace)
        results = res.results
        global LAST_EXEC_NS
        LAST_EXEC_NS = res.exec_time_ns
        if trace:
            print(f"HW exec time: {res.exec_time_ns} ns")
    out = np.empty((cfg.N, cfg.D), dtype=np.float32)
    for c in range(cfg.NC):
        out[c * cfg.R : (c + 1) * cfg.R] = results[c]["out"][: cfg.R]
    return out


# revision 60
# speedup vs baseline: 1.0591x; 1.0591x over previous
"""GCN (2-layer) SpMM kernel for 8 TRN2 NeuronCores via Bass/Tile.

Strategy (1D row partitioning, per sharding hint):
  - Destination rows sharded across 8 cores (12500 rows/core, padded to 12544).
  - support1 = x @ W1 computed fully (all nodes) on every core locally,
    stored as an fp16 row-major table in HBM in "padded row space"
    (node n -> row m(n)).  x is supplied pre-transposed + quad-interleaved so
    phase 1 needs no DMA transpose and stores in 512B-contiguous runs.
  - SpMM per layer: edges of a core (contiguous, adj_row sorted) are grouped
    by (window of 128 dest rows, quad-pair parity, row parity) on the host and
    padded to 128-edge tiles.  Per gather batch (<=8 tiles):
      * dma_gather pulls 256B "node pair" elements (2 adjacent fp16 rows,
        at 512B quad stride) from the table into SBUF (edges on partitions),
      * two DVE tensor_tensor ops build all selection matrices of the batch
        at once: S[p, t, j] = val[p,t] * (iota[j] == rowloc[p,t]),
      * one PE matmul per tile accumulates S^T @ G_half into its window PSUM.
    Gather indices are preloaded into SBUF in bulk (no per-gather DMA);
    trailing padding carries idx=-1 so the Q7 firmware skips those
    descriptors (interior padding gathers row 0 with val=0).  An explicit
    depth-8 gather ring keeps ~8 gathers in flight, matching the 4 SWDGE
    queues' ring capacity; a depth-3 ring costs +1.9ms.
  - Bias is added via a K=1 matmul (ones^T @ b) opening each PSUM window.
  - Layer-1 window close: LeakyReLU (ACT Lrelu), PE transpose, h @ W2 ->
    support2 shard; AllGather (fp16) makes the full support2 table; layer 2
    repeats the SpMM and writes fp32 output rows.

Self-contained: hardcodes all shapes; only needs the staged runtime
(concourse) available on the machine, as provided in this container.
"""

import os
import numpy as np

# ---------------------------------------------------------------- config ---


class Cfg:
    def __init__(self, N, E, D=64, NC=8, W=128, SUPER=2, MG=8, IC=8192, QT=8,
                 NQ=4):
        self.N, self.E, self.D, self.NC, self.W, self.SUPER = N, E, D, NC, W, SUPER
        self.R = N // NC                      # real rows per core
        self.NW = -(-self.R // W)             # windows per core
        self.NW = -(-self.NW // SUPER) * SUPER
        self.NSG = self.NW // SUPER
        self.RP = self.NW * W                 # padded rows per core
        self.NPAD = self.RP * NC              # padded table rows
        assert self.NPAD % 512 == 0
        self.NQUAD = self.NPAD // 4           # gather idx = quad index
        assert self.NQUAD <= 32767            # int16 idx
        self.QP = 2                           # pair-within-quad
        self.PAR = 2                          # row-within-pair
        self.MG = MG                          # max tiles per gather instr
        self.IC = IC                          # idx SBUF buffer columns
        self.QT = QT                          # phase-1 512-row blocks per batch
        self.NQ = NQ                          # SWDGE queues
        # MG>8 needs single_packet=False (64-desc packet cap), which makes
        # every 256B descriptor its own packet: measured +0.55ms gpsimd and
        # +0.4ms SWDGE drain at MG=24/NQ=2.  MG=8 + single_packet is best.
        self.RPB = self.RP // 512             # phase-1 full blocks per shard
        self.RTQ = (self.RP % 512) // 4       # tail quads (one 64-quad block)
        assert self.RP % 512 in (0, 256)

    def m_of_node(self, n):
        """node id -> padded table row"""
        return (n // self.R) * self.RP + (n % self.R)


FULL = Cfg(N=100000, E=3200000)
LAST_EXEC_NS = None


# ------------------------------------------------------------- host prep ---


def schedule(cfg, adj_row, adj_col, adj_val):
    """Group each core's edges by (supergroup, quad-parity, row-parity,
    window) and pad each group to whole 128-edge tiles with a schedule shared
    across cores (max tile count per group)."""
    N, NC, W, SUPER = cfg.N, cfg.NC, cfg.W, cfg.SUPER
    QP, PAR, NSG, R = cfg.QP, cfg.PAR, cfg.NSG, cfg.R

    row = np.asarray(adj_row, dtype=np.int64)
    col = np.asarray(adj_col, dtype=np.int64)
    val = np.asarray(adj_val, dtype=np.float32)

    bounds = np.searchsorted(row, np.arange(NC + 1) * R)
    cores = []
    ngroups = NSG * QP * PAR * SUPER
    counts = np.zeros((NC, ngroups), dtype=np.int64)
    for c in range(NC):
        e0, e1 = bounds[c], bounds[c + 1]
        r = (row[e0:e1] - c * R).astype(np.int64)
        m = cfg.m_of_node(col[e0:e1])
        v = val[e0:e1]
        w = r // W
        rowloc = r % W
        quad = m >> 2
        qp = (m >> 1) & 1
        par = m & 1
        sg = w // SUPER
        w4 = w % SUPER
        # emission order: for sg: for qp: for (par, w4) tiles
        key = ((sg * QP + qp) * PAR + par) * SUPER + w4
        order = np.argsort(key, kind="stable")
        cores.append(
            dict(key=key[order], qidx=quad[order], rowloc=rowloc[order], v=v[order])
        )
        counts[c] = np.bincount(key, minlength=ngroups)

    gtiles = -(-counts.max(axis=0) // 128)  # [ngroups]
    tiles = gtiles.reshape(NSG, QP, PAR, SUPER)
    T = max(int(gtiles.sum()), 1)
    return tiles, counts, cores, T


def plan_batches(cfg, tiles):
    """Gather-batch plan, shared by host prep and device program.

    Returns (batches, runs).  Each batch: dict(sg, qp, t0, pc, tmap) where
    tmap[tl] = (par, w4) for each tile in the batch.  Each run: (rt0, rt1)
    tile range whose idx columns are bulk-loaded into one SBUF buffer.
    """
    NSG, QP, PAR, SUPER, MG = cfg.NSG, cfg.QP, cfg.PAR, cfg.SUPER, cfg.MG
    batches = []
    t0 = 0
    for sg in range(NSG):
        for qp in range(QP):
            tmap = []
            for par in range(PAR):
                for w4 in range(SUPER):
                    tmap += [(par, w4)] * int(tiles[sg, qp, par, w4])
            cnt = len(tmap)
            for p0 in range(0, cnt, MG):
                pc = min(MG, cnt - p0)
                batches.append(
                    dict(sg=sg, qp=qp, t0=t0 + p0, pc=pc, tmap=tmap[p0 : p0 + pc])
                )
            t0 += cnt
    # pack batches into idx-buffer runs of <= IC columns (8 cols per tile)
    cap_tiles = cfg.IC // 8
    runs = []
    rt0 = 0
    cur = 0
    for b in batches:
        if b["t0"] + b["pc"] - rt0 > cap_tiles:
            runs.append((rt0, b["t0"]))
            rt0 = b["t0"]
        b["run"] = len(runs)
        cur = b["t0"] + b["pc"]
    runs.append((rt0, max(cur, rt0 + 1)))
    return batches, runs


def prep(cfg, adj_row, adj_col, adj_val):
    """Build the shared tile schedule + per-core edge streams.

    Returns (tiles, batches, runs, per_core list of dicts with
    idxrep [128, T*8] int16, rowloc [128, T] fp16, val [128, T] fp16, T).
    """
    tiles, counts, cores, T = schedule(cfg, adj_row, adj_col, adj_val)
    batches, runs = plan_batches(cfg, tiles)
    gtiles = tiles.reshape(-1)
    ngroups = gtiles.shape[0]
    tstart = np.concatenate([[0], np.cumsum(gtiles)])

    streams = []
    reals = []
    for c in range(cfg.NC):
        d = cores[c]
        idx_s = np.zeros(T * 128, dtype=np.int16)
        rl_s = np.zeros(T * 128, dtype=np.float16)
        vl_s = np.zeros(T * 128, dtype=np.float16)
        real = np.zeros(T * 128, dtype=bool)
        gstart = np.concatenate([[0], np.cumsum(counts[c])])
        for g in range(ngroups):
            cnt = int(counts[c][g])
            if cnt == 0:
                continue
            s0, t0 = gstart[g], tstart[g] * 128
            idx_s[t0 : t0 + cnt] = d["qidx"][s0 : s0 + cnt]
            rl_s[t0 : t0 + cnt] = d["rowloc"][s0 : s0 + cnt]
            vl_s[t0 : t0 + cnt] = d["v"][s0 : s0 + cnt]
            real[t0 : t0 + cnt] = True
        streams.append((idx_s, rl_s, vl_s))
        reals.append(real)

    # Trailing padding of each gather batch -> idx -1: the Q7 firmware trims
    # trailing negatives, skipping their descriptor generation and DMA.
    # num_idxs_reg must equal the non-negative count and is compile-time
    # shared across cores, so the -1 run starts at the max-over-cores last
    # real slot.  Skipped SBUF slots read stale-but-finite data (the gather
    # ring buffers are memset once at start); S has val=0 there.
    for b in batches:
        lo, hi = b["t0"] * 128, (b["t0"] + b["pc"]) * 128
        valid = 1
        for c in range(cfg.NC):
            nz = np.nonzero(reals[c][lo:hi])[0]
            if nz.size:
                valid = max(valid, int(nz[-1]) + 1)
        b["valid"] = valid
        for c in range(cfg.NC):
            streams[c][0][lo + valid : hi] = -1

    per_core = []
    for c in range(cfg.NC):
        idx_s, rl_s, vl_s = streams[c]
        per_core.append(
            dict(
                idxrep=np.broadcast_to(
                    idx_s.reshape(-1, 16).T, (8, 16, T * 8)
                ).reshape(128, T * 8)
                .copy(),                                   # [128, T*8]
                rowloc=rl_s.reshape(T, 128).T.copy(),      # [128, T] fp16
                val=vl_s.reshape(T, 128).T.copy(),         # [128, T] fp16
            )
        )
    return tiles, batches, runs, per_core, T


# --------------------------------------------------------- device program ---


def build_program(cfg, tiles, batches, runs, T, lrelu_native=True):
    import concourse.bass as bass
    import concourse.bacc as bacc
    from concourse import mybir
    from concourse.tile import TileContext

    f16, f32, i16 = mybir.dt.float16, mybir.dt.float32, mybir.dt.int16
    D, W, SUPER, QP, PAR, NSG = cfg.D, cfg.W, cfg.SUPER, cfg.QP, cfg.PAR, cfg.NSG
    NPAD, RP, NQUAD, QT = cfg.NPAD, cfg.RP, cfg.NQUAD, cfg.QT
    SLOTS = T * 8
    RPB, RTQ = cfg.RPB, cfg.RTQ  # per-shard phase-1 blocks

    nc = bacc.Bacc(num_devices=cfg.NC, num_swdge_queues=cfg.NQ,
                   dynamic_dma_scratch_size=65536)

    xtq = nc.declare_dram_parameter("xtq", [D, RP], f16, isOutput=False)
    w1p = nc.declare_dram_parameter("w1", [D, D], f16, isOutput=False)
    w2p = nc.declare_dram_parameter("w2", [D, D], f16, isOutput=False)
    b1p = nc.declare_dram_parameter("b1", [1, D], f16, isOutput=False)
    b2p = nc.declare_dram_parameter("b2", [1, D], f16, isOutput=False)
    idxp = nc.declare_dram_parameter("idxrep", [128, SLOTS], i16, isOutput=False)
    rlp = nc.declare_dram_parameter("rowloc", [128, T], f16, isOutput=False)
    vlp = nc.declare_dram_parameter("val", [128, T], f16, isOutput=False)
    outp = nc.declare_dram_parameter("out", [RP, D], f32, isOutput=True)

    sup1sh = nc.dram_tensor("sup1sh", [RP, D], f16)
    sup1 = nc.dram_tensor("sup1", [NPAD, D], f16, addr_space="Shared")
    s2sh = nc.dram_tensor("s2sh", [RP, D], f16)
    s2full = nc.dram_tensor("s2full", [NPAD, D], f16, addr_space="Shared")

    eq = mybir.AluOpType.is_equal
    mult = mybir.AluOpType.mult

    with TileContext(nc) as tc:
        with (
            tc.tile_pool(name="const", bufs=1) as cp,
            tc.tile_pool(name="meta", bufs=1) as mp,
        ):
            w1s = cp.tile([D, D], f16, tag="w1")
            nc.sync.dma_start(out=w1s[:], in_=w1p[:])
            w2s = cp.tile([D, D], f16, tag="w2")
            nc.sync.dma_start(out=w2s[:], in_=w2p[:])
            b1s = cp.tile([1, D], f16, tag="b1")
            nc.sync.dma_start(out=b1s[:], in_=b1p[:])
            b2s = cp.tile([1, D], f16, tag="b2")
            nc.sync.dma_start(out=b2s[:], in_=b2p[:])
            ones = cp.tile([1, W], f16, tag="ones")
            nc.vector.memset(ones[:], 1.0)
            iota = cp.tile([128, W], f16, tag="iota")
            nc.gpsimd.iota(
                iota[:], [[1, W]], channel_multiplier=0,
                allow_small_or_imprecise_dtypes=True,
            )
            iotap = cp.tile([128, 1], f32, tag="iotap")
            nc.gpsimd.iota(
                iotap[:], [[1, 1]], channel_multiplier=1,
                allow_small_or_imprecise_dtypes=True,
            )
            ident = cp.tile([128, 128], f16, tag="ident")
            nc.vector.tensor_scalar(
                out=ident[:], in0=iota[:, 0:128], scalar1=iotap[:], scalar2=None,
                op0=eq,
            )
            rls = mp.tile([128, T], f16, tag="rl")
            nc.sync.dma_start(out=rls[:], in_=rlp[:])
            vls = mp.tile([128, T], f16, tag="vl")
            nc.sync.dma_start(out=vls[:], in_=vlp[:])

            # ---------------- phase 1: support1 = x @ W1 (own shard) ------
            # Each core computes only its own RP table rows, then an
            # AllGather assembles the full table.  x arrives transposed +
            # quad-interleaved per shard: column b*512 + r*128 + p holds the
            # x row of shard row (b*128 + p)*4 + r, so each psum evacuates
            # into 512B-contiguous quad rows.
            with (
                tc.tile_pool(name="ph1x", bufs=2) as xp,
                tc.tile_pool(name="ph1s", bufs=2) as stp,
                tc.tile_pool(name="ph1ps", bufs=4, space="PSUM") as pp1,
            ):
                for qb in range(0, RPB, QT):
                    nb = min(QT, RPB - qb)
                    xt = xp.tile([D, 512 * nb], f16, tag="xt")
                    nc.sync.dma_start(
                        out=xt[:], in_=xtq[:, qb * 512 : (qb + nb) * 512]
                    )
                    st = stp.tile([128, nb, 256], f16, tag="st")
                    for jj in range(nb):
                        for r4 in range(4):
                            ps = pp1.tile([128, D], f32, tag="ps")
                            nc.tensor.matmul(
                                ps[:],
                                lhsT=xt[:, jj * 512 + r4 * 128 : jj * 512 + (r4 + 1) * 128],
                                rhs=w1s[:],
                                start=True, stop=True,
                            )
                            nc.scalar.activation(
                                out=st[:, jj, r4 * D : (r4 + 1) * D], in_=ps[:],
                                func=mybir.ActivationFunctionType.Copy,
                            )
                    nc.sync.dma_start(
                        out=sup1sh[qb * 512 : (qb + nb) * 512, :].rearrange(
                            "(b p x) d -> p b (x d)", p=128, x=4
                        ),
                        in_=st[:],
                    )
                if RTQ:
                    # 256-row tail block: 64 quads, col r*64 + p
                    xt = xp.tile([D, 4 * RTQ], f16, tag="xtt")
                    nc.sync.dma_start(out=xt[:], in_=xtq[:, RPB * 512 :])
                    st = stp.tile([RTQ, 4 * D], f16, tag="stt")
                    for r4 in range(4):
                        ps = pp1.tile([RTQ, D], f32, tag="pst")
                        nc.tensor.matmul(
                            ps[:],
                            lhsT=xt[:, r4 * RTQ : (r4 + 1) * RTQ],
                            rhs=w1s[:],
                            start=True, stop=True,
                        )
                        nc.scalar.activation(
                            out=st[:, r4 * D : (r4 + 1) * D], in_=ps[:],
                            func=mybir.ActivationFunctionType.Copy,
                        )
                    nc.sync.dma_start(
                        out=sup1sh[RPB * 512 :, :].rearrange(
                            "(p x) d -> p (x d)", x=4
                        ),
                        in_=st[:],
                    )
            nc.gpsimd.collective_compute(
                "AllGather",
                mybir.AluOpType.bypass,
                replica_groups=[list(range(cfg.NC))],
                ins=[sup1sh[:]],
                outs=[sup1[:]],
            )

            # ---------------- SpMM layers --------------------------------
            with (
                tc.tile_pool(name="ib", bufs=2) as ibp,
                tc.tile_pool(name="gp", bufs=8) as gp,
                tc.tile_pool(name="sp", bufs=3) as sp,
                tc.tile_pool(name="hp", bufs=3) as hp,
                tc.tile_pool(name="op", bufs=3) as op,
                tc.tile_pool(name="accp", bufs=5, space="PSUM") as accp,
                tc.tile_pool(name="ptp", bufs=1, space="PSUM") as ptp,
                tc.tile_pool(name="ps2p", bufs=1, space="PSUM") as ps2p,
            ):
                # explicit gather ring: fixed buffers, zeroed once, so slots
                # skipped by trailing -1 idxs always read finite data
                # explicit depth-8 gather ring (same depth as the best pool
                # config, which is what the earlier depth-3 trimming attempt
                # lacked): fixed buffers, zeroed once, so slots skipped by
                # trailing -1 idxs always read finite data
                NGB = 8
                gits = []
                for i in range(NGB):
                    g = gp.tile(
                        [128, cfg.MG, 2 * D], f16, tag=f"G{i}", bufs=1,
                        name=f"gring{i}",
                    )
                    nc.vector.memset(g[:], 0.0)
                    gits.append(g)

                def spmm_layer(layer, table, bias_s):
                    gq = [0]
                    cur_run = [-1, None]  # run id, tile handle

                    def get_idx(b):
                        rid = b["run"]
                        if cur_run[0] != rid:
                            rt0, rt1 = runs[rid]
                            ncols = (rt1 - rt0) * 8
                            ibt = ibp.tile([128, ncols], i16, tag="ib")
                            nc.sync.dma_start(
                                out=ibt[:], in_=idxp[:, rt0 * 8 : rt1 * 8]
                            )
                            cur_run[0], cur_run[1] = rid, ibt
                        rt0 = runs[rid][0]
                        off = (b["t0"] - rt0) * 8
                        # pass only ceil(valid/16) idx columns: the Q7 idx
                        # repack loop scales with static num_idxs, so a tight
                        # num_idxs also trims repack work, not just descgen
                        return cur_run[1][:, off : off + -(-b["valid"] // 16)]

                    bi = 0
                    for sg in range(NSG):
                        left = [
                            int(tiles[sg, :, :, w4].sum()) for w4 in range(SUPER)
                        ]
                        psums = []
                        for w4 in range(SUPER):
                            ps = accp.tile([W, D], f32, tag="acc")
                            nc.tensor.matmul(
                                ps[:], lhsT=ones[:], rhs=bias_s[:],
                                start=True, stop=(left[w4] == 0),
                            )
                            psums.append(ps)
                        while bi < len(batches) and batches[bi]["sg"] == sg:
                            b = batches[bi]
                            pc, qp, t0 = b["pc"], b["qp"], b["t0"]
                            idx_ap = get_idx(b)
                            git = gits[gq[0] % NGB][:, :pc, :]
                            nc.gpsimd.dma_gather(
                                git,
                                bass.AP(
                                    table, qp * 2 * D,
                                    [[4 * D, NQUAD], [1, 2 * D]],
                                ),
                                idx_ap,
                                num_idxs=b["valid"],
                                num_idxs_reg=b["valid"],
                                elem_size=2 * D,
                                elem_step=4 * D,
                                single_packet=cfg.MG <= 8,
                                queue_num=gq[0] % cfg.NQ,
                            )
                            gq[0] += 1
                            seq = sp.tile([128, pc, W], f16, tag="Seq")
                            nc.vector.tensor_tensor(
                                out=seq[:],
                                in0=iota[:].unsqueeze(1).broadcast_to([128, pc, W]),
                                in1=rls[:, t0 : t0 + pc]
                                .unsqueeze(2)
                                .broadcast_to([128, pc, W]),
                                op=eq,
                            )
                            Ss = sp.tile([128, pc, W], f16, tag="S")
                            nc.vector.tensor_tensor(
                                out=Ss[:],
                                in0=seq[:],
                                in1=vls[:, t0 : t0 + pc]
                                .unsqueeze(2)
                                .broadcast_to([128, pc, W]),
                                op=mult,
                            )
                            for tl in range(pc):
                                par, w4 = b["tmap"][tl]
                                left[w4] -= 1
                                nc.tensor.matmul(
                                    psums[w4][:],
                                    lhsT=Ss[:, tl, :],
                                    rhs=git[:, tl, par * D : par * D + D],
                                    start=False,
                                    stop=(left[w4] == 0),
                                )
                            bi += 1
                        # window close
                        if layer == 0:
                            s2t = hp.tile([W, SUPER, D], f16, tag="s2t")
                        else:
                            s2t = op.tile([W, SUPER, D], f32, tag="outt")
                        for w4 in range(SUPER):
                            if layer == 0:
                                hh = hp.tile([W, D], f16, tag="hh")
                                if lrelu_native:
                                    nc.scalar.activation(
                                        out=hh[:], in_=psums[w4][:],
                                        func=mybir.ActivationFunctionType.Lrelu,
                                        alpha=0.2,
                                    )
                                else:
                                    hpos = hp.tile([W, D], f16, tag="hpos")
                                    nc.scalar.activation(
                                        out=hpos[:], in_=psums[w4][:],
                                        func=mybir.ActivationFunctionType.Relu,
                                    )
                                    hneg = hp.tile([W, D], f16, tag="hneg")
                                    nc.vector.tensor_scalar(
                                        out=hneg[:], in0=psums[w4][:],
                                        scalar1=0.0, scalar2=0.2,
                                        op0=mybir.AluOpType.min, op1=mult,
                                    )
                                    nc.vector.tensor_tensor(
                                        out=hh[:], in0=hpos[:], in1=hneg[:],
                                        op=mybir.AluOpType.add,
                                    )
                                pt = ptp.tile([D, W], f16, tag="pt")
                                nc.tensor.transpose(pt[:], hh[:], ident[:])
                                hT = hp.tile([D, W], f16, tag="hT")
                                nc.scalar.activation(
                                    out=hT[:], in_=pt[:],
                                    func=mybir.ActivationFunctionType.Copy,
                                )
                                ps2 = ps2p.tile([W, D], f32, tag="ps2")
                                nc.tensor.matmul(
                                    ps2[:], lhsT=hT[:], rhs=w2s[:],
                                    start=True, stop=True,
                                )
                                nc.scalar.activation(
                                    out=s2t[:, w4, :], in_=ps2[:],
                                    func=mybir.ActivationFunctionType.Copy,
                                )
                            else:
                                nc.scalar.activation(
                                    out=s2t[:, w4, :], in_=psums[w4][:],
                                    func=mybir.ActivationFunctionType.Copy,
                                )
                        dst = s2sh if layer == 0 else outp
                        nc.sync.dma_start(
                            out=dst[
                                sg * SUPER * W : (sg + 1) * SUPER * W, :
                            ].rearrange("(t p) d -> p t d", p=W),
                            in_=s2t[:],
                        )

                spmm_layer(0, sup1, b1s)
                nc.gpsimd.collective_compute(
                    "AllGather",
                    mybir.AluOpType.bypass,
                    replica_groups=[list(range(cfg.NC))],
                    ins=[s2sh[:]],
                    outs=[s2full[:]],
                )
                spmm_layer(1, s2full, b2s)

    nc.compile()
    return nc


# ----------------------------------------------------------------- kernel ---


def make_inputs(cfg, x, adj_row, adj_col, adj_val, W1, b1, W2, b2):
    tiles, batches, runs, per_core, T = prep(cfg, adj_row, adj_col, adj_val)
    x = np.asarray(x, dtype=np.float32)
    xpad = np.zeros((cfg.NPAD, cfg.D), dtype=np.float16)
    xpad[cfg.m_of_node(np.arange(cfg.N))] = x.astype(np.float16)
    common = dict(
        w1=np.asarray(W1, np.float16),
        w2=np.asarray(W2, np.float16),
        b1=np.asarray(b1, np.float16).reshape(1, cfg.D),
        b2=np.asarray(b2, np.float16).reshape(1, cfg.D),
    )
    in_maps = []
    for c in range(cfg.NC):
        m = dict(common)
        # shard-local transpose + quad interleave: col b*512 + r*128 + p
        # (tail: RPB*512 + r*RTQ + p) <- shard row (quad)*4 + r
        xl = xpad[c * cfg.RP : (c + 1) * cfg.RP]
        xf = (
            xl[: cfg.RPB * 512]
            .reshape(cfg.RPB, 128, 4, cfg.D)
            .transpose(0, 2, 1, 3)
            .reshape(-1, cfg.D)
        )
        if cfg.RTQ:
            xt = (
                xl[cfg.RPB * 512 :]
                .reshape(cfg.RTQ, 4, cfg.D)
                .transpose(1, 0, 2)
                .reshape(-1, cfg.D)
            )
            xf = np.concatenate([xf, xt], axis=0)
        m["xtq"] = np.ascontiguousarray(xf.T)      # [D, RP]
        m["idxrep"] = per_core[c]["idxrep"]
        m["rowloc"] = per_core[c]["rowloc"]
        m["val"] = per_core[c]["val"]
        in_maps.append(m)
    return tiles, batches, runs, in_maps, T


def kernel(x, adj_row, adj_col, adj_val, W1, b1, W2, b2, _cfg=None, _sim=False):
    cfg = _cfg or FULL
    tiles, batches, runs, in_maps, T = make_inputs(
        cfg, x, adj_row, adj_col, adj_val, W1, b1, W2, b2
    )
    # Native ACT Lrelu mis-evaluates on HW (rel err 0.2 observed); keep the
    # Relu + min/mult + add composition on both paths.
    nc = build_program(cfg, tiles, batches, runs, T, lrelu_native=False)
    if _sim:
        from concourse import bass_interp

        sim = bass_interp.MultiCoreSim(nc, cfg.NC)
        for c in range(cfg.NC):
            for k, v in in_maps[c].items():
                sim.cores[c].tensor(k)[:] = v
        sim.simulate()
        results = [{"out": np.array(sim.cores[c].tensor("out"))} for c in range(cfg.NC)]
    else:
        from concourse.bass_utils import run_bass_kernel_spmd

        trace = bool(int(os.environ.get("GCN_TRACE", "0")))
        res = run_bass_kernel_spmd(nc, in_maps, list(range(cfg.NC)), trace=trace)
        results = res.results
        global LAST_EXEC_NS
        LAST_EXEC_NS = res.exec_time_ns
        if trace:
            print(f"HW exec time: {res.exec_time_ns} ns")
    out = np.empty((cfg.N, cfg.D), dtype=np.float32)
    for c in range(cfg.NC):
        out[c * cfg.R : (c + 1) * cfg.R] = results[c]["out"][: cfg.R]
    return out
